# revision 65
# baseline (speedup 1.0000x reference)
"""Biased multi-head attention block (LayerNorm -> QKV -> attn+bias softmax -> out proj)
on 8 Trainium2 NeuronCores, data-parallel over the batch dimension (one batch element
per core).

Per-core device kernel layout strategy:
  - LayerNorm in [token, dim] layout (bn_stats/bn_aggr + tensor_scalar with a
    bf16 output), then PE transpose (bf16, 1 cycle/row) to xnTb [dim, token];
    all 8 transposes of a tile land in one 1-bank PSUM tile and are evicted
    with a single strided copy (per-instruction DVE dispatch ~300ns dwarfs a
    [128,128] copy).
  - V is projected in two passes of 4 token tiles (kt-outer, feature-half
    split so each accumulator is one PSUM bank), from SBUF-resident bf16 wv.
    V lands in [token, feat] layout with an extra all-ones column per head, so
    the attention row-sums (softmax denominators) fall out of the same matmul
    that computes attn @ V.
  - Q,K are projected into qT/kT [feat, token] in PURE BF16 (bf16 weights
    stationary, bf16 xnTb moving): bf16 LDWEIGHTS (~107ns) hides completely
    under the previous matmul's 512-column fill, so the PE issues matmuls at
    the N-cycle floor — f32r weight loads (~185ns + NX serialization) do not.
    Each head pair's projection matmuls are slotted into the PREVIOUS pair's
    attention steps, filling PE bubbles so the tensor engine stays dense
    (keeps the HAM clock gate at 2.4 GHz).
  - Attention is computed transposed per head: simT[j, i] = k_h^T q_h on PSUM
    (bf16 operands, fp32 accumulate); exp() on the scalar engine directly out
    of PSUM with the 1/8 head scale folded into the activation's affine
    prestep; the additive attention bias becomes a vector-engine multiply by
    host-precomputed exp(bias)^T in bf16 (exp(a+b) = exp(a)exp(b)).
  - attn_exp^T is the moving operand of outT_h = [v_h|1]^T @ expT.
  - Softmax normalization is PER HEAD PAIR: each pair's row sums are
    reciprocal'd and broadcast over the pair's 128 outT rows by a tiny
    [2,128] selection matmul, slotted into the NEXT pair's attention — no
    global end-of-attention barrier, so the PE stays warm straight into the
    output projection.
  - Final projection y = outT^T @ w_out with outT stationary; kt=7 is
    accumulated last so the first psy tiles never wait on pair 7's norm.
  - EVERY matmul runs in bf16 end-to-end (fp32 PSUM accumulation): bf16
    weight loads hide under 512-column fills, which f32r loads cannot.
  - DMA triggers are engine-load-balanced: the Sync queue serializes
    triggers at ~600ns each, so the 128 attention-bias loads ride the
    otherwise-idle GPSIMD queue; w_v (2MB bf16) is fully SBUF-resident and
    loaded behind the first two x tiles (trigger order is transfer priority).
  - PSUM is phase-scoped: LayerNorm/V uses 1-bank transpose-batch and V
    accumulator tiles (2+4+2 banks); attention uses 4x 2-bank slots.

Measured on hardware: ~304-308us exec per core (8 cores in parallel),
rel err ~6.8e-3 vs the fp32 reference (session baseline: 419-435us, 3e-3).
"""

import os

import numpy as np
import ml_dtypes

import concourse.bacc as bacc
import concourse.bass as bass
import concourse.mybir as mybir
import concourse.tile as tile
from concourse.bass_utils import run_bass_kernel_spmd
from concourse.masks import make_identity

B = 8
N = 1024
DIM = 1024
HEADS = 16
DH = 64
INNER = HEADS * DH
P = 128
NT = N // P          # token tiles
KT = DIM // P        # contraction tiles
PAIRS = HEADS // 2   # head pairs (one qT/kT feature tile each)
EPS = 1e-5
SCALE = DH ** -0.5   # 0.125, exact in fp32

F32 = mybir.dt.float32
BF16 = mybir.dt.bfloat16
AF = mybir.ActivationFunctionType

_BUILD_CACHE = {}


def _maybe_enable_ldw_opt():
    """Opt-in (known-broken for f32r): rewrite walrus args so LDWEIGHTS can use
    the background weight buffer. bass_utils hardcodes --enable-ldw-opt=false;
    intercept its run_command to flip it."""
    if not bool(int(os.environ.get("BA_LDW_OPT", "0"))):
        return
    import concourse.bass_utils as _bu

    if getattr(_bu.run_command, "_ldw_patched", False):
        return
    _orig = _bu.run_command

    def _patched(argv, **kwargs):
        argv = [
            a.replace("--enable-ldw-opt=false", "--enable-ldw-opt=true")
            if isinstance(a, str)
            else a
            for a in argv
        ]
        return _orig(argv, **kwargs)

    _patched._ldw_patched = True
    _bu.run_command = _patched


def _build(apply_gamma: bool, apply_beta: bool):
    key = (apply_gamma, apply_beta)
    if key in _BUILD_CACHE:
        return _BUILD_CACHE[key]
    _maybe_enable_ldw_opt()

    nc = bacc.Bacc("TRN2", target_bir_lowering=False, debug=False)
    if bool(int(os.environ.get("BA_LDW_OPT", "0"))):
        # Walrus's LDW background-buffer opt rejects standalone InstLdweights.
        # The tile scheduler emits them as bare prefetch hints (no syncs)
        # ahead of matmuls that still self-load their weights, and
        # move_matmul_waits_to_ldweights parks extra waits on them. Drop the
        # hints and keep waits on the matmuls; generate_event_semaphores
        # splits any excess into EventSemaphore instructions instead.
        nc.move_matmul_waits_to_ldweights = lambda: None
        _orig_compile = nc.compile

        def _compile_without_ldw_hints():
            for blk in nc.main_func.blocks:
                keep = []
                pending_sync = []
                for inst in blk.instructions:
                    if isinstance(inst, mybir.InstLdweights):
                        if inst.sync_info is not None:
                            pending_sync.append(inst.sync_info)
                        continue
                    if pending_sync and isinstance(inst, mybir.InstMatmult):
                        si = inst.sync_info
                        if si is None:
                            si = mybir.SyncInfo(on_wait=[], on_update=[])
                            inst.sync_info = si
                        for ps in pending_sync:
                            si.on_wait.extend(ps.on_wait)
                            si.on_update.extend(ps.on_update)
                        pending_sync = []
                    keep.append(inst)
                assert not pending_sync
                blk.instructions[:] = keep
            _orig_compile()

        nc.compile = _compile_without_ldw_hints

    x_d = nc.dram_tensor("x", [N, DIM], F32, kind="ExternalInput")
    wqk_d = nc.dram_tensor("wqk", [PAIRS, P, KT, 2 * P], BF16, kind="ExternalInput")
    wv_d = nc.dram_tensor("wv", [P, KT, DIM], BF16, kind="ExternalInput")
    wo_d = nc.dram_tensor("wo", [P, KT, DIM], BF16, kind="ExternalInput")
    bias_d = nc.dram_tensor("biasT", [HEADS, NT, P, N], BF16, kind="ExternalInput")
    sel8_d = nc.dram_tensor("sel8", [8, 4, P], BF16, kind="ExternalInput")
    gamma_d = beta_d = None
    if apply_gamma:
        gamma_d = nc.dram_tensor("gamma", [DIM], F32, kind="ExternalInput")
    if apply_beta:
        beta_d = nc.dram_tensor("beta", [DIM], F32, kind="ExternalInput")
    y_d = nc.dram_tensor("y", [N, DIM], F32, kind="ExternalOutput")

    with tile.TileContext(nc) as tc:
        from contextlib import ExitStack

        with ExitStack() as ctx:
            consts = ctx.enter_context(tc.tile_pool(name="consts", bufs=1))
            xpool = ctx.enter_context(tc.tile_pool(name="xpool", bufs=4))
            stats = ctx.enter_context(tc.tile_pool(name="stats", bufs=4))
            bigp = ctx.enter_context(tc.tile_pool(name="bigp", bufs=1))
            vpool = ctx.enter_context(tc.tile_pool(name="vpool", bufs=NT))
            wstream = ctx.enter_context(tc.tile_pool(name="wstream", bufs=3))
            qkpool = ctx.enter_context(tc.tile_pool(name="qkpool", bufs=4))
            epool = ctx.enter_context(tc.tile_pool(name="epool", bufs=6))
            bpool = ctx.enter_context(tc.tile_pool(name="bpool", bufs=6))
            opool = ctx.enter_context(tc.tile_pool(name="opool", bufs=KT))

            ident = consts.tile([P, P], BF16, name="ident")
            make_identity(nc, ident)
            eps_t = consts.tile([P, 1], F32, name="eps_t")
            nc.vector.memset(eps_t, EPS)
            # Selection matrix for the per-pair softmax normalization.
            # The pair's reciprocal row sums live in an [8, 256] tile
            # (partition = head*4 + column-chunk) so the DVE reciprocal runs
            # on 8 lanes instead of 2; sel8[:, c, :] broadcasts chunk c over
            # the pair's 128 outT feature rows (head 0 of pair = rows 0:64).
            sel8 = consts.tile([8, 4, P], BF16, name="sel8")
            nc.sync.dma_start(out=sel8, in_=sel8_d[:, :, :])

            gamma_t = beta_t = None
            if apply_gamma:
                gamma_t = consts.tile([P, DIM], F32, name="gamma_t")
                g_ap = gamma_d[:]
                nc.sync.dma_start(
                    out=gamma_t,
                    in_=bass.AP(
                        tensor=g_ap.tensor, offset=g_ap.offset, ap=[[0, P]] + list(g_ap.ap)
                    ),
                )
            if apply_beta:
                beta_t = consts.tile([P, DIM], F32, name="beta_t")
                b_ap = beta_d[:]
                nc.sync.dma_start(
                    out=beta_t,
                    in_=bass.AP(
                        tensor=b_ap.tensor, offset=b_ap.offset, ap=[[0, P]] + list(b_ap.ap)
                    ),
                )

            xnTb = bigp.tile([P, KT, N], BF16, name="xnTb", tag="bigb")
            # w_v is small in bf16 (2MB) — keep it fully resident. One DMA
            # trigger instead of 32 chunk loads: the Sync engine serializes
            # DMA triggers at ~600ns each, and the V phase was stalling on
            # trigger latency, not bandwidth.
            # w_v is loaded after the first two x tiles (trigger order is
            # transfer priority: DMA rings round-robin, so anything issued
            # before x0 delays the whole LayerNorm chain). Two chunks so V's
            # first kt matmuls don't wait for the full 2MB.
            wvfull = bigp.tile([P, KT, DIM], BF16, name="wvfull", tag="bigv")

            vts = []
            for jt in range(NT):
                vt = vpool.tile([P, HEADS * (DH + 1)], BF16, name=f"v{jt}", tag="v")
                vv = vt.rearrange("p (h c) -> p h c", c=DH + 1)
                vts.append((vt, vv))

            # ---- Phases A+B1: LayerNorm + V projection --------------------
            # Phase-scoped PSUM pool: transpose batches are 1-bank bf16 tiles
            # (3 bufs) and the V accumulators are 1-bank [128,512] fp32 tiles
            # (4 bufs, feature-half split), so next-half transposes never
            # starve while V accumulates — 7 of 8 banks, no slot contention.
            psA = tc.alloc_tile_pool(name="psA", bufs=1, space="PSUM")

            def emit_ln(it):
                xt = xpool.tile([P, DIM], F32, name=f"x{it}", tag="x")
                nc.sync.dma_start(out=xt, in_=x_d[it * P : (it + 1) * P, :])
                st = stats.tile([P, 2, 6], F32, name=f"st{it}", tag="st")
                nc.vector.bn_stats(out=st[:, 0], in_=xt[:, 0:512])
                nc.vector.bn_stats(out=st[:, 1], in_=xt[:, 512:1024])
                mv = stats.tile([P, 2], F32, name=f"mv{it}", tag="mv")
                nc.vector.bn_aggr(out=mv, in_=st)
                std = stats.tile([P, 1], F32, name=f"sd{it}", tag="sd")
                nc.scalar.activation(out=std, in_=mv[:, 1:2], func=AF.Sqrt, bias=eps_t)
                rstd = stats.tile([P, 1], F32, name=f"rs{it}", tag="rs")
                nc.vector.reciprocal(out=rstd, in_=std)
                xtb = xpool.tile([P, DIM], BF16, name=f"xb{it}", tag="xb")
                if gamma_t is None and beta_t is None:
                    nc.vector.tensor_scalar(
                        out=xtb,
                        in0=xt,
                        scalar1=mv[:, 0:1],
                        scalar2=rstd,
                        op0=mybir.AluOpType.subtract,
                        op1=mybir.AluOpType.mult,
                    )
                else:
                    nc.vector.tensor_scalar(
                        out=xt,
                        in0=xt,
                        scalar1=mv[:, 0:1],
                        scalar2=rstd,
                        op0=mybir.AluOpType.subtract,
                        op1=mybir.AluOpType.mult,
                    )
                    if gamma_t is not None and beta_t is not None:
                        nc.vector.tensor_mul(xt, xt, gamma_t)
                        nc.vector.tensor_add(xtb, xt, beta_t)
                    elif gamma_t is not None:
                        nc.vector.tensor_mul(xtb, xt, gamma_t)
                    else:
                        nc.vector.tensor_add(xtb, xt, beta_t)
                # All 8 transposes of this tile land in ONE 1-bank PSUM tile,
                # evicted with a single strided copy — per-instruction DVE
                # dispatch (~300ns) dwarfs the copy itself at [128,128].
                pt = psA.tile([P, KT * P], BF16, name=f"tp{it}", tag="pt", bufs=2)
                for kt in range(KT):
                    nc.tensor.transpose(
                        pt[:, kt * P : (kt + 1) * P],
                        xtb[:, kt * P : (kt + 1) * P],
                        ident,
                    )
                nc.vector.tensor_copy(
                    xnTb[:, :, it * P : (it + 1) * P],
                    pt.rearrange("p (k c) -> p k c", c=P),
                )

            wqs, qTs, kTs = [], [], []
            wq0 = wstream.tile([P, KT, 2 * P], BF16, name="wqk0", tag="w")
            wqs.append(wq0)
            qTs.append(qkpool.tile([P, N], BF16, name="qT0", tag="qk"))
            kTs.append(qkpool.tile([P, N], BF16, name="kT0", tag="qk"))

            def proj_steps(pn, which, pool, tag):
                """Generator: 16 matmuls (kt-outer, hf-inner — the two hf
                matmuls share one stationary wq block, halving LDWEIGHTS
                weight switches) accumulating pair pn's q (which=0) or k
                (which=1) projection in pure bf16, then evicting per half.
                Yields after each matmul; with 3 slotted per attention step
                the eviction still lands ~2 steps before the pair boundary."""
                ps = pool.tile([P, N], F32, name=f"ps{'qk'[which]}{pn}", tag=tag)
                w0 = which * P
                dst = (qTs, kTs)[which][pn]
                for kt in range(KT):
                    for hf in range(2):
                        sl = slice(hf * 512, hf * 512 + 512)
                        nc.tensor.matmul(
                            ps[:, sl],
                            lhsT=wqs[pn][:, kt, w0 : w0 + P],
                            rhs=xnTb[:, kt, sl],
                            start=(kt == 0),
                            stop=(kt == KT - 1),
                        )
                        yield
                for hf in range(2):
                    sl = slice(hf * 512, hf * 512 + 512)
                    nc.vector.tensor_copy(dst[:, sl], ps[:, sl])
                while True:
                    yield

            # Two passes of 4 token tiles, each split into feature halves, so
            # the V accumulators are 1-bank tiles. Pair 0's q/k projections
            # (the attention prologue) are slotted into the second pass's V
            # matmul stream — they cost no extra wall-clock.
            for half in range(2):
                for j in range(4):
                    emit_ln(half * 4 + j)
                if half == 0:
                    nc.sync.dma_start(out=wvfull[:, 0:4, :], in_=wv_d[:, 0:4, :])
                    nc.sync.dma_start(out=wvfull[:, 4:8, :], in_=wv_d[:, 4:8, :])
                    nc.sync.dma_start(out=wq0, in_=wqk_d[0])
                for fh in range(2):
                    proj = (
                        proj_steps(0, fh, psA, "pp")
                        if half == 1
                        else iter(())
                    )
                    psv = [
                        psA.tile(
                            [P, 512], F32, name=f"psv{half}{fh}_{j}",
                            tag="psv", bufs=4,
                        )
                        for j in range(4)
                    ]
                    for kt in range(KT):
                        for j in range(4):
                            jt = half * 4 + j
                            nc.tensor.matmul(
                                psv[j],
                                lhsT=xnTb[:, kt, jt * P : (jt + 1) * P],
                                rhs=wvfull[:, kt, fh * 512 : fh * 512 + 512],
                                start=(kt == 0),
                                stop=(kt == KT - 1),
                            )
                        next(proj, None)
                        next(proj, None)
                    for _ in range(4):
                        next(proj, None)
                    for j in range(4):
                        jt = half * 4 + j
                        nc.vector.tensor_copy(
                            vts[jt][1][:, fh * 8 : fh * 8 + 8, 0:DH], psv[j]
                        )
            psA.release()
            # One shared PSUM pool for everything after LayerNorm/V: 4 slots
            # sized [128, 1024] fp32 (2 banks each = all 8 banks).
            pspool = ctx.enter_context(
                tc.tile_pool(name="pspool", bufs=4, space="PSUM")
            )

            # The all-ones columns that produce the softmax row sums; emitted
            # here so the 16 small memsets sit in the DVE queue after the
            # LayerNorm chains, not ahead of them.
            for jt in range(NT):
                nc.vector.memset(vts[jt][1][:, :, DH : DH + 1], 1.0)

            s2s, r2s, outTs = [], [], []
            wo_t = bigp.tile([P, KT, DIM], BF16, name="wo_t", tag="big")

            def emit_norm(p):
                """Normalize pair p's outT block by its softmax row sums:
                one [8,256] reciprocal + four 256-wide sel8 broadcast matmuls
                + one elementwise multiply. Slotted late into pair p+1's
                attention so the PE never waits on the reciprocal."""
                with nc.allow_low_precision(
                    reason="recip feeds an fp32r matmul; fp32r rounding intended"
                ):
                    nc.vector.reciprocal(out=r2s[p], in_=s2s[p])
                rs = pspool.tile([P, N], F32, name=f"rsn{p}", tag="ps")
                for c in range(4):
                    nc.tensor.matmul(
                        rs[:, c * 256 : (c + 1) * 256],
                        lhsT=sel8[:, c, :],
                        rhs=r2s[p],
                        start=True,
                        stop=True,
                    )
                nc.vector.tensor_mul(outTs[p], outTs[p], rs)

            for p in range(PAIRS):
                qT, kTt = qTs[p], kTs[p]
                if p + 1 < PAIRS:
                    wq = wstream.tile(
                        [P, KT, 2 * P], BF16, name=f"wqk{p+1}", tag="w"
                    )
                    nc.sync.dma_start(out=wq, in_=wqk_d[p + 1])
                    wqs.append(wq)
                    qTs.append(
                        qkpool.tile([P, N], BF16, name=f"qT{p+1}", tag="qk")
                    )
                    kTs.append(
                        qkpool.tile([P, N], BF16, name=f"kT{p+1}", tag="qk")
                    )

                ot = opool.tile([P, N], BF16, name=f"outT{p}", tag="outT")
                outTs.append(ot)
                # Trickle one kt-slice of w_out per pair so the 4MB load is
                # spread across the attention phase (the sync queue drains
                # DMA triggers in program order — issuing all of wo at phase
                # F would park it behind every bias load).
                nc.sync.dma_start(out=wo_t[:, p, :], in_=wo_d[:, p, :])
                s2s.append(stats.tile([8, 256], BF16, name=f"s2_{p}", tag="s2"))
                r2s.append(stats.tile([8, 256], BF16, name=f"r2_{p}", tag="r2"))
                avs_t = [None, None]
                ets = [[], []]

                def emit_sim(hh, jt):
                    h = 2 * p + hh
                    hs = slice(hh * DH, (hh + 1) * DH)
                    bt = bpool.tile([P, N], BF16, name=f"b{h}_{jt}", tag="bias")
                    # Bias loads ride the (otherwise idle) GPSIMD queue — the
                    # Sync queue serializes DMA triggers at ~600ns each and
                    # these 128 loads were adding latency to every small DMA.
                    nc.gpsimd.dma_start(out=bt, in_=bias_d[h, jt])
                    sim = pspool.tile([P, N], F32, name=f"sim{h}_{jt}", tag="ps")
                    for hf in range(2):
                        sl = slice(hf * 512, hf * 512 + 512)
                        nc.tensor.matmul(
                            sim[:, sl],
                            lhsT=kTt[hs, jt * P : (jt + 1) * P],
                            rhs=qT[hs, sl],
                            start=True,
                            stop=True,
                        )
                    et = epool.tile([P, N], BF16, name=f"e{h}_{jt}", tag="exp")
                    nc.scalar.activation(out=et, in_=sim, func=AF.Exp, scale=SCALE)
                    nc.vector.tensor_mul(et, et, bt)
                    ets[hh].append(et)

                def emit_av(hh, jt):
                    h = 2 * p + hh
                    for hf in range(2):
                        sl = slice(hf * 512, hf * 512 + 512)
                        nc.tensor.matmul(
                            avs_t[hh][:, sl],
                            lhsT=vts[jt][0][:, h * (DH + 1) : (h + 1) * (DH + 1)],
                            rhs=ets[hh][jt][:, sl],
                            start=(jt == 0),
                            stop=(jt == NT - 1),
                        )

                def emit_evict(hh):
                    h = 2 * p + hh
                    hs = slice(hh * DH, (hh + 1) * DH)
                    # Evict via SBUF staging (DMA cannot read PSUM; DVE cannot
                    # shift partitions — stage on matching partitions, then DMA
                    # to the head's row block in outT and its row of the
                    # pair's sums tile). The sums row is copied FIRST so the
                    # reciprocal chain starts before the 64-row block copy.
                    avs = xpool.tile([DH + 1, N], BF16, name=f"avs{h}", tag="avs")
                    nc.vector.tensor_copy(
                        avs[DH : DH + 1, :], avs_t[hh][DH : DH + 1, :]
                    )
                    # Head hh's 1024 sums land as 4 chunks of 256 on
                    # partitions 4*hh .. 4*hh+3 (chunk-major fill).
                    nc.sync.dma_start(
                        out=s2s[p][4 * hh : 4 * hh + 4, :],
                        in_=avs[DH : DH + 1, :],
                    )
                    nc.vector.tensor_copy(avs[0:DH, :], avs_t[hh][0:DH, :])
                    nc.sync.dma_start(out=outTs[p][hs, :], in_=avs[0:DH, :])

                for hh in range(2):
                    avs_t[hh] = pspool.tile(
                        [DH + 1, N], F32, name=f"av{2*p+hh}", tag="ps"
                    )
                    # head 0 drives pair p+1's q projection, head 1 its k
                    # projection: matmuls slotted into each jt step.
                    proj = (
                        proj_steps(p + 1, hh, pspool, "ps")
                        if p + 1 < PAIRS
                        else iter(())
                    )

                    def proj_step(k=3):
                        for _ in range(k):
                            next(proj, None)

                    emit_sim(hh, 0)
                    proj_step()
                    for jt in range(1, NT):
                        emit_sim(hh, jt)
                        emit_av(hh, jt - 1)
                        proj_step()
                        if hh == 1 and jt == 3 and p > 0:
                            emit_norm(p - 1)
                    emit_av(hh, NT - 1)
                    proj_step(1)  # run the projection eviction
                    emit_evict(hh)

            # ---- Phase F: y = outT^T @ w_out ------------------------------
            # Three psy groups accumulate kt=0..6 (21 matmuls, ~4.5us) while
            # pair 7's reciprocal chain completes, then its norm runs, then
            # the held kt=7 contributions — the PE never idles >3.4us, so no
            # HAM re-throttle into the tail. (Only 3 groups: the norm's rs
            # tile needs the 4th PSUM slot.)
            psys = {}
            ysts = {}

            def psy_mm(it, hf, kt):
                sl = slice(hf * 512, hf * 512 + 512)
                nc.tensor.matmul(
                    psys[(it, hf)],
                    lhsT=outTs[kt][:, it * P : (it + 1) * P],
                    rhs=wo_t[:, kt, sl],
                    start=(kt == 0),
                    stop=(kt == KT - 1),
                )

            prelude = [(0, 0), (0, 1), (1, 0)]
            for it in range(2):
                ysts[it] = xpool.tile([P, DIM], F32, name=f"y{it}", tag="x")
            for it, hf in prelude:
                psys[(it, hf)] = pspool.tile(
                    [P, 512], F32, name=f"psy{it}_{hf}", tag="ps"
                )
                for kt in range(KT - 1):
                    psy_mm(it, hf, kt)
            emit_norm(PAIRS - 1)
            for it, hf in prelude:
                psy_mm(it, hf, KT - 1)
                nc.vector.tensor_copy(
                    ysts[it][:, hf * 512 : hf * 512 + 512], psys[(it, hf)]
                )
            for it in range(NT):
                if it >= 2:
                    ysts[it] = xpool.tile([P, DIM], F32, name=f"y{it}", tag="x")
                for hf in range(2):
                    if (it, hf) in psys:
                        continue
                    sl = slice(hf * 512, hf * 512 + 512)
                    psys[(it, hf)] = pspool.tile(
                        [P, 512], F32, name=f"psy{it}_{hf}", tag="ps"
                    )
                    for kt in range(KT):
                        psy_mm(it, hf, kt)
                    nc.vector.tensor_copy(ysts[it][:, sl], psys[(it, hf)])
                nc.sync.dma_start(out=y_d[it * P : (it + 1) * P, :], in_=ysts[it])

    nc.compile()
    _BUILD_CACHE[key] = nc
    return nc


def _host_prep(ln_gamma, ln_beta, w_qkv, w_out, attn_bias):
    """Re-layout weights/bias for the device kernel (pure host-side reshapes)."""
    w_qkv = np.asarray(w_qkv, np.float32)
    w_out = np.asarray(w_out, np.float32)
    attn_bias = np.asarray(attn_bias, np.float32)

    wq_r = w_qkv[:, :INNER].reshape(KT, P, PAIRS, P).transpose(2, 1, 0, 3)
    wk_r = w_qkv[:, INNER : 2 * INNER].reshape(KT, P, PAIRS, P).transpose(2, 1, 0, 3)
    wqk = np.ascontiguousarray(
        np.concatenate([wq_r, wk_r], axis=3).astype(ml_dtypes.bfloat16)
    )
    wv = np.ascontiguousarray(
        w_qkv[:, 2 * INNER :].reshape(KT, P, DIM).transpose(1, 0, 2)
        .astype(ml_dtypes.bfloat16)
    )
    wo = np.ascontiguousarray(
        w_out.reshape(KT, P, DIM).transpose(1, 0, 2).astype(ml_dtypes.bfloat16)
    )
    # exp(bias), transposed per head to [j, i]: the kernel multiplies it into
    # exp(sim) on the vector engine (exp(a+b) = exp(a)*exp(b)), keeping the
    # tensor engine free of bias-add matmuls.
    biasT = np.ascontiguousarray(
        np.exp(attn_bias[0].astype(np.float64)).astype(np.float32)
        .transpose(0, 2, 1)
        .reshape(HEADS, NT, P, N)
        .astype(ml_dtypes.bfloat16)
    )
    sel8 = np.zeros((8, 4, P), dtype=ml_dtypes.bfloat16)
    for c in range(4):
        sel8[c, c, 0:DH] = 1.0        # head 0 of the pair -> outT rows 0:64
        sel8[4 + c, c, DH:P] = 1.0    # head 1 of the pair -> outT rows 64:128
    in_map = {"wqk": wqk, "wv": wv, "wo": wo, "biasT": biasT, "sel8": sel8}

    gamma = np.asarray(ln_gamma, np.float32)
    beta = np.asarray(ln_beta, np.float32)
    apply_gamma = not np.all(gamma == 1.0)
    apply_beta = bool(np.any(beta != 0.0))
    if apply_gamma:
        in_map["gamma"] = gamma
    if apply_beta:
        in_map["beta"] = beta
    return in_map, apply_gamma, apply_beta


def kernel(x, ln_gamma, ln_beta, w_qkv, w_out, attn_bias):
    x = np.asarray(x, np.float32)
    in_map, apply_gamma, apply_beta = _host_prep(
        ln_gamma, ln_beta, w_qkv, w_out, attn_bias
    )
    nc = _build(apply_gamma, apply_beta)
    in_maps = [dict(in_map, x=np.ascontiguousarray(x[b])) for b in range(B)]
    res = run_bass_kernel_spmd(
        nc,
        in_maps,
        list(range(B)),
        trace=bool(int(os.environ.get("BA_TRACE", "0"))),
        tmpdir=os.environ.get("BA_TRACE_DIR") or None,
    )
    out = np.stack([res.results[i]["y"] for i in range(B)], axis=0)
    if bool(int(os.environ.get("BA_TRACE", "0"))):
        kernel.last_exec_time_ns = res.exec_time_ns
        kernel.last_mean_exec_time_ns = res.mean_exec_time_ns
    return out


# revision 67
# speedup vs baseline: 1.0100x; 1.0100x over previous
"""Biased multi-head attention block (LayerNorm -> QKV -> attn+bias softmax -> out proj)
on 8 Trainium2 NeuronCores, data-parallel over the batch dimension (one batch element
per core).

Per-core device kernel layout strategy:
  - LayerNorm in [token, dim] layout (bn_stats/bn_aggr + tensor_scalar with a
    bf16 output), then PE transpose (bf16, 1 cycle/row) to xnTb [dim, token];
    all 8 transposes of a tile land in one 1-bank PSUM tile and are evicted
    with a single strided copy (per-instruction DVE dispatch ~300ns dwarfs a
    [128,128] copy).
  - V is projected in two passes of 4 token tiles (kt-outer, feature-half
    split so each accumulator is one PSUM bank), from SBUF-resident bf16 wv.
    V lands in [token, feat] layout with an extra all-ones column per head, so
    the attention row-sums (softmax denominators) fall out of the same matmul
    that computes attn @ V.
  - Q,K are projected into qT/kT [feat, token] in PURE BF16 (bf16 weights
    stationary, bf16 xnTb moving): bf16 LDWEIGHTS (~107ns) hides completely
    under the previous matmul's 512-column fill, so the PE issues matmuls at
    the N-cycle floor — f32r weight loads (~185ns + NX serialization) do not.
    Each head pair's projection matmuls are slotted into the PREVIOUS pair's
    attention steps, filling PE bubbles so the tensor engine stays dense
    (keeps the HAM clock gate at 2.4 GHz).
  - Attention is computed transposed per head: simT[j, i] = k_h^T q_h on PSUM
    (bf16 operands, fp32 accumulate); exp() on the scalar engine directly out
    of PSUM with the 1/8 head scale folded into the activation's affine
    prestep; the additive attention bias becomes a vector-engine multiply by
    host-precomputed exp(bias)^T in bf16 (exp(a+b) = exp(a)exp(b)).
  - attn_exp^T is the moving operand of outT_h = [v_h|1]^T @ expT.
  - Softmax normalization is PER HEAD PAIR: each pair's row sums are
    reciprocal'd and broadcast over the pair's 128 outT rows by a tiny
    [2,128] selection matmul, slotted into the NEXT pair's attention — no
    global end-of-attention barrier, so the PE stays warm straight into the
    output projection.
  - Final projection y = outT^T @ w_out with outT stationary; kt=7 is
    accumulated last so the first psy tiles never wait on pair 7's norm.
  - EVERY matmul runs in bf16 end-to-end (fp32 PSUM accumulation): bf16
    weight loads hide under 512-column fills, which f32r loads cannot.
  - DMA triggers are engine-load-balanced: the Sync queue serializes
    triggers at ~600ns each, so the 128 attention-bias loads ride the
    otherwise-idle GPSIMD queue; w_v (2MB bf16) is fully SBUF-resident and
    loaded behind the first two x tiles (trigger order is transfer priority).
  - PSUM is phase-scoped: LayerNorm/V uses 1-bank transpose-batch and V
    accumulator tiles (2+4+2 banks); attention uses 4x 2-bank slots.

Measured on hardware: ~304-308us exec per core (8 cores in parallel),
rel err ~6.8e-3 vs the fp32 reference (session baseline: 419-435us, 3e-3).
"""

import os

import numpy as np
import ml_dtypes

import concourse.bacc as bacc
import concourse.bass as bass
import concourse.mybir as mybir
import concourse.tile as tile
from concourse.bass_utils import run_bass_kernel_spmd
from concourse.masks import make_identity

B = 8
N = 1024
DIM = 1024
HEADS = 16
DH = 64
INNER = HEADS * DH
P = 128
NT = N // P          # token tiles
KT = DIM // P        # contraction tiles
PAIRS = HEADS // 2   # head pairs (one qT/kT feature tile each)
EPS = 1e-5
SCALE = DH ** -0.5   # 0.125, exact in fp32

F32 = mybir.dt.float32
BF16 = mybir.dt.bfloat16
AF = mybir.ActivationFunctionType

_BUILD_CACHE = {}


def _maybe_enable_ldw_opt():
    """Opt-in (known-broken for f32r): rewrite walrus args so LDWEIGHTS can use
    the background weight buffer. bass_utils hardcodes --enable-ldw-opt=false;
    intercept its run_command to flip it."""
    if not bool(int(os.environ.get("BA_LDW_OPT", "0"))):
        return
    import concourse.bass_utils as _bu

    if getattr(_bu.run_command, "_ldw_patched", False):
        return
    _orig = _bu.run_command

    def _patched(argv, **kwargs):
        argv = [
            a.replace("--enable-ldw-opt=false", "--enable-ldw-opt=true")
            if isinstance(a, str)
            else a
            for a in argv
        ]
        return _orig(argv, **kwargs)

    _patched._ldw_patched = True
    _bu.run_command = _patched


def _build(apply_gamma: bool, apply_beta: bool):
    key = (apply_gamma, apply_beta)
    if key in _BUILD_CACHE:
        return _BUILD_CACHE[key]
    _maybe_enable_ldw_opt()

    nc = bacc.Bacc("TRN2", target_bir_lowering=False, debug=False)
    if bool(int(os.environ.get("BA_LDW_OPT", "0"))):
        # Walrus's LDW background-buffer opt rejects standalone InstLdweights.
        # The tile scheduler emits them as bare prefetch hints (no syncs)
        # ahead of matmuls that still self-load their weights, and
        # move_matmul_waits_to_ldweights parks extra waits on them. Drop the
        # hints and keep waits on the matmuls; generate_event_semaphores
        # splits any excess into EventSemaphore instructions instead.
        nc.move_matmul_waits_to_ldweights = lambda: None
        _orig_compile = nc.compile

        def _compile_without_ldw_hints():
            for blk in nc.main_func.blocks:
                keep = []
                pending_sync = []
                for inst in blk.instructions:
                    if isinstance(inst, mybir.InstLdweights):
                        if inst.sync_info is not None:
                            pending_sync.append(inst.sync_info)
                        continue
                    if pending_sync and isinstance(inst, mybir.InstMatmult):
                        si = inst.sync_info
                        if si is None:
                            si = mybir.SyncInfo(on_wait=[], on_update=[])
                            inst.sync_info = si
                        for ps in pending_sync:
                            si.on_wait.extend(ps.on_wait)
                            si.on_update.extend(ps.on_update)
                        pending_sync = []
                    keep.append(inst)
                assert not pending_sync
                blk.instructions[:] = keep
            _orig_compile()

        nc.compile = _compile_without_ldw_hints

    x_d = nc.dram_tensor("x", [N, DIM], F32, kind="ExternalInput")
    wqk_d = nc.dram_tensor("wqk", [PAIRS, P, KT, 2 * P], BF16, kind="ExternalInput")
    wv_d = nc.dram_tensor("wv", [P, KT, DIM], BF16, kind="ExternalInput")
    wo_d = nc.dram_tensor("wo", [P, KT, DIM], BF16, kind="ExternalInput")
    bias_d = nc.dram_tensor("biasT", [HEADS, NT, P, N], BF16, kind="ExternalInput")
    sel8_d = nc.dram_tensor("sel8", [8, 4, P], BF16, kind="ExternalInput")
    gamma_d = beta_d = None
    if apply_gamma:
        gamma_d = nc.dram_tensor("gamma", [DIM], F32, kind="ExternalInput")
    if apply_beta:
        beta_d = nc.dram_tensor("beta", [DIM], F32, kind="ExternalInput")
    y_d = nc.dram_tensor("y", [N, DIM], F32, kind="ExternalOutput")

    with tile.TileContext(nc) as tc:
        from contextlib import ExitStack

        with ExitStack() as ctx:
            consts = ctx.enter_context(tc.tile_pool(name="consts", bufs=1))
            xpool = ctx.enter_context(tc.tile_pool(name="xpool", bufs=4))
            stats = ctx.enter_context(tc.tile_pool(name="stats", bufs=4))
            bigp = ctx.enter_context(tc.tile_pool(name="bigp", bufs=1))
            vpool = ctx.enter_context(tc.tile_pool(name="vpool", bufs=NT))
            wstream = ctx.enter_context(tc.tile_pool(name="wstream", bufs=3))
            qkpool = ctx.enter_context(tc.tile_pool(name="qkpool", bufs=4))
            epool = ctx.enter_context(tc.tile_pool(name="epool", bufs=6))
            bpool = ctx.enter_context(tc.tile_pool(name="bpool", bufs=6))
            opool = ctx.enter_context(tc.tile_pool(name="opool", bufs=KT))

            ident = consts.tile([P, P], BF16, name="ident")
            make_identity(nc, ident)
            eps_t = consts.tile([P, 1], F32, name="eps_t")
            nc.vector.memset(eps_t, EPS)
            # Selection matrix for the per-pair softmax normalization.
            # The pair's reciprocal row sums live in an [8, 256] tile
            # (partition = head*4 + column-chunk) so the DVE reciprocal runs
            # on 8 lanes instead of 2; sel8[:, c, :] broadcasts chunk c over
            # the pair's 128 outT feature rows (head 0 of pair = rows 0:64).
            sel8 = consts.tile([8, 4, P], BF16, name="sel8")
            nc.sync.dma_start(out=sel8, in_=sel8_d[:, :, :])

            gamma_t = beta_t = None
            if apply_gamma:
                gamma_t = consts.tile([P, DIM], F32, name="gamma_t")
                g_ap = gamma_d[:]
                nc.sync.dma_start(
                    out=gamma_t,
                    in_=bass.AP(
                        tensor=g_ap.tensor, offset=g_ap.offset, ap=[[0, P]] + list(g_ap.ap)
                    ),
                )
            if apply_beta:
                beta_t = consts.tile([P, DIM], F32, name="beta_t")
                b_ap = beta_d[:]
                nc.sync.dma_start(
                    out=beta_t,
                    in_=bass.AP(
                        tensor=b_ap.tensor, offset=b_ap.offset, ap=[[0, P]] + list(b_ap.ap)
                    ),
                )

            xnTb = bigp.tile([P, KT, N], BF16, name="xnTb", tag="bigb")
            # w_v is small in bf16 (2MB) — keep it fully resident. One DMA
            # trigger instead of 32 chunk loads: the Sync engine serializes
            # DMA triggers at ~600ns each, and the V phase was stalling on
            # trigger latency, not bandwidth.
            # w_v is loaded after the first two x tiles (trigger order is
            # transfer priority: DMA rings round-robin, so anything issued
            # before x0 delays the whole LayerNorm chain). Two chunks so V's
            # first kt matmuls don't wait for the full 2MB.
            wvfull = bigp.tile([P, KT, DIM], BF16, name="wvfull", tag="bigv")

            vts = []
            for jt in range(NT):
                vt = vpool.tile([P, HEADS * (DH + 1)], BF16, name=f"v{jt}", tag="v")
                vv = vt.rearrange("p (h c) -> p h c", c=DH + 1)
                vts.append((vt, vv))

            # ---- Phases A+B1: LayerNorm + V projection --------------------
            # Phase-scoped PSUM pool: transpose batches are 1-bank bf16 tiles
            # (3 bufs) and the V accumulators are 1-bank [128,512] fp32 tiles
            # (4 bufs, feature-half split), so next-half transposes never
            # starve while V accumulates — 7 of 8 banks, no slot contention.
            psA = tc.alloc_tile_pool(name="psA", bufs=1, space="PSUM")

            def emit_ln(it):
                xt = xpool.tile([P, DIM], F32, name=f"x{it}", tag="x")
                # Two half loads: bn_stats on columns 0:512 starts as soon as
                # the first 256KB lands instead of waiting for the full tile.
                nc.sync.dma_start(
                    out=xt[:, 0:512], in_=x_d[it * P : (it + 1) * P, 0:512]
                )
                nc.sync.dma_start(
                    out=xt[:, 512:1024], in_=x_d[it * P : (it + 1) * P, 512:1024]
                )
                st = stats.tile([P, 2, 6], F32, name=f"st{it}", tag="st")
                nc.vector.bn_stats(out=st[:, 0], in_=xt[:, 0:512])
                nc.vector.bn_stats(out=st[:, 1], in_=xt[:, 512:1024])
                mv = stats.tile([P, 2], F32, name=f"mv{it}", tag="mv")
                nc.vector.bn_aggr(out=mv, in_=st)
                std = stats.tile([P, 1], F32, name=f"sd{it}", tag="sd")
                nc.scalar.activation(out=std, in_=mv[:, 1:2], func=AF.Sqrt, bias=eps_t)
                rstd = stats.tile([P, 1], F32, name=f"rs{it}", tag="rs")
                nc.vector.reciprocal(out=rstd, in_=std)
                xtb = xpool.tile([P, DIM], BF16, name=f"xb{it}", tag="xb")
                if gamma_t is None and beta_t is None:
                    nc.vector.tensor_scalar(
                        out=xtb,
                        in0=xt,
                        scalar1=mv[:, 0:1],
                        scalar2=rstd,
                        op0=mybir.AluOpType.subtract,
                        op1=mybir.AluOpType.mult,
                    )
                else:
                    nc.vector.tensor_scalar(
                        out=xt,
                        in0=xt,
                        scalar1=mv[:, 0:1],
                        scalar2=rstd,
                        op0=mybir.AluOpType.subtract,
                        op1=mybir.AluOpType.mult,
                    )
                    if gamma_t is not None and beta_t is not None:
                        nc.vector.tensor_mul(xt, xt, gamma_t)
                        nc.vector.tensor_add(xtb, xt, beta_t)
                    elif gamma_t is not None:
                        nc.vector.tensor_mul(xtb, xt, gamma_t)
                    else:
                        nc.vector.tensor_add(xtb, xt, beta_t)
                # All 8 transposes of this tile land in ONE 1-bank PSUM tile,
                # evicted with a single strided copy — per-instruction DVE
                # dispatch (~300ns) dwarfs the copy itself at [128,128].
                pt = psA.tile([P, KT * P], BF16, name=f"tp{it}", tag="pt", bufs=2)
                for kt in range(KT):
                    nc.tensor.transpose(
                        pt[:, kt * P : (kt + 1) * P],
                        xtb[:, kt * P : (kt + 1) * P],
                        ident,
                    )
                nc.vector.tensor_copy(
                    xnTb[:, :, it * P : (it + 1) * P],
                    pt.rearrange("p (k c) -> p k c", c=P),
                )

            wqs, qTs, kTs = [], [], []
            wq0 = wstream.tile([P, KT, 2 * P], BF16, name="wqk0", tag="w")
            wqs.append(wq0)
            qTs.append(qkpool.tile([P, N], BF16, name="qT0", tag="qk"))
            kTs.append(qkpool.tile([P, N], BF16, name="kT0", tag="qk"))

            def proj_steps(pn, which, pool, tag):
                """Generator: 16 matmuls (hf-outer, kt-inner) accumulating
                pair pn's q (which=0) or k (which=1) projection in pure bf16.
                Each feature half is evicted to SBUF as soon as its 8-matmul
                accumulation stops, so the next pair's first sim never waits
                on a whole-projection eviction. Yields after each matmul."""
                ps = pool.tile([P, N], F32, name=f"ps{'qk'[which]}{pn}", tag=tag)
                w0 = which * P
                dst = (qTs, kTs)[which][pn]
                for hf in range(2):
                    sl = slice(hf * 512, hf * 512 + 512)
                    for kt in range(KT):
                        nc.tensor.matmul(
                            ps[:, sl],
                            lhsT=wqs[pn][:, kt, w0 : w0 + P],
                            rhs=xnTb[:, kt, sl],
                            start=(kt == 0),
                            stop=(kt == KT - 1),
                        )
                        yield
                    nc.vector.tensor_copy(dst[:, sl], ps[:, sl])
                while True:
                    yield

            # Two passes of 4 token tiles, each split into feature halves, so
            # the V accumulators are 1-bank tiles. Pair 0's q/k projections
            # (the attention prologue) are slotted into the second pass's V
            # matmul stream — they cost no extra wall-clock.
            for half in range(2):
                for j in range(4):
                    emit_ln(half * 4 + j)
                if half == 0:
                    nc.sync.dma_start(out=wvfull[:, 0:4, :], in_=wv_d[:, 0:4, :])
                    nc.sync.dma_start(out=wvfull[:, 4:8, :], in_=wv_d[:, 4:8, :])
                    nc.sync.dma_start(out=wq0, in_=wqk_d[0])
                for fh in range(2):
                    proj = (
                        proj_steps(0, fh, psA, "pp")
                        if half == 1
                        else iter(())
                    )
                    psv = [
                        psA.tile(
                            [P, 512], F32, name=f"psv{half}{fh}_{j}",
                            tag="psv", bufs=4,
                        )
                        for j in range(4)
                    ]
                    for kt in range(KT):
                        for j in range(4):
                            jt = half * 4 + j
                            nc.tensor.matmul(
                                psv[j],
                                lhsT=xnTb[:, kt, jt * P : (jt + 1) * P],
                                rhs=wvfull[:, kt, fh * 512 : fh * 512 + 512],
                                start=(kt == 0),
                                stop=(kt == KT - 1),
                            )
                        next(proj, None)
                        next(proj, None)
                    for _ in range(4):
                        next(proj, None)
                    for j in range(4):
                        jt = half * 4 + j
                        nc.vector.tensor_copy(
                            vts[jt][1][:, fh * 8 : fh * 8 + 8, 0:DH], psv[j]
                        )
            psA.release()
            # One shared PSUM pool for everything after LayerNorm/V: 4 slots
            # sized [128, 1024] fp32 (2 banks each = all 8 banks).
            pspool = ctx.enter_context(
                tc.tile_pool(name="pspool", bufs=4, space="PSUM")
            )

            # The all-ones columns that produce the softmax row sums; emitted
            # here so the 16 small memsets sit in the DVE queue after the
            # LayerNorm chains, not ahead of them.
            for jt in range(NT):
                nc.vector.memset(vts[jt][1][:, :, DH : DH + 1], 1.0)

            s2s, r2s, outTs = [], [], []
            wo_t = bigp.tile([P, KT, DIM], BF16, name="wo_t", tag="big")

            def emit_norm(p):
                """Normalize pair p's outT block by its softmax row sums:
                one [8,256] reciprocal + four 256-wide sel8 broadcast matmuls
                + one elementwise multiply. Slotted late into pair p+1's
                attention so the PE never waits on the reciprocal."""
                with nc.allow_low_precision(
                    reason="recip feeds an fp32r matmul; fp32r rounding intended"
                ):
                    nc.vector.reciprocal(out=r2s[p], in_=s2s[p])
                rs = pspool.tile([P, N], F32, name=f"rsn{p}", tag="ps")
                for c in range(4):
                    nc.tensor.matmul(
                        rs[:, c * 256 : (c + 1) * 256],
                        lhsT=sel8[:, c, :],
                        rhs=r2s[p],
                        start=True,
                        stop=True,
                    )
                nc.vector.tensor_mul(outTs[p], outTs[p], rs)

            for p in range(PAIRS):
                qT, kTt = qTs[p], kTs[p]
                if p + 1 < PAIRS:
                    wq = wstream.tile(
                        [P, KT, 2 * P], BF16, name=f"wqk{p+1}", tag="w"
                    )
                    nc.sync.dma_start(out=wq, in_=wqk_d[p + 1])
                    wqs.append(wq)
                    qTs.append(
                        qkpool.tile([P, N], BF16, name=f"qT{p+1}", tag="qk")
                    )
                    kTs.append(
                        qkpool.tile([P, N], BF16, name=f"kT{p+1}", tag="qk")
                    )

                ot = opool.tile([P, N], BF16, name=f"outT{p}", tag="outT")
                outTs.append(ot)
                # Trickle one kt-slice of w_out per pair so the 4MB load is
                # spread across the attention phase (the sync queue drains
                # DMA triggers in program order — issuing all of wo at phase
                # F would park it behind every bias load).
                nc.sync.dma_start(out=wo_t[:, p, :], in_=wo_d[:, p, :])
                s2s.append(stats.tile([8, 256], BF16, name=f"s2_{p}", tag="s2"))
                r2s.append(stats.tile([8, 256], BF16, name=f"r2_{p}", tag="r2"))
                avs_t = [None, None]
                ets = [[], []]

                def emit_sim(hh, jt):
                    h = 2 * p + hh
                    hs = slice(hh * DH, (hh + 1) * DH)
                    bt = bpool.tile([P, N], BF16, name=f"b{h}_{jt}", tag="bias")
                    # Bias loads ride the (otherwise idle) GPSIMD queue — the
                    # Sync queue serializes DMA triggers at ~600ns each and
                    # these 128 loads were adding latency to every small DMA.
                    nc.gpsimd.dma_start(out=bt, in_=bias_d[h, jt])
                    sim = pspool.tile([P, N], F32, name=f"sim{h}_{jt}", tag="ps")
                    for hf in range(2):
                        sl = slice(hf * 512, hf * 512 + 512)
                        nc.tensor.matmul(
                            sim[:, sl],
                            lhsT=kTt[hs, jt * P : (jt + 1) * P],
                            rhs=qT[hs, sl],
                            start=True,
                            stop=True,
                        )
                    et = epool.tile([P, N], BF16, name=f"e{h}_{jt}", tag="exp")
                    nc.scalar.activation(out=et, in_=sim, func=AF.Exp, scale=SCALE)
                    nc.vector.tensor_mul(et, et, bt)
                    ets[hh].append(et)

                def emit_av(hh, jt):
                    h = 2 * p + hh
                    for hf in range(2):
                        sl = slice(hf * 512, hf * 512 + 512)
                        nc.tensor.matmul(
                            avs_t[hh][:, sl],
                            lhsT=vts[jt][0][:, h * (DH + 1) : (h + 1) * (DH + 1)],
                            rhs=ets[hh][jt][:, sl],
                            start=(jt == 0),
                            stop=(jt == NT - 1),
                        )

                def emit_evict(hh):
                    h = 2 * p + hh
                    hs = slice(hh * DH, (hh + 1) * DH)
                    # Evict via SBUF staging (DMA cannot read PSUM; DVE cannot
                    # shift partitions — stage on matching partitions, then DMA
                    # to the head's row block in outT and its row of the
                    # pair's sums tile). The sums row is copied FIRST so the
                    # reciprocal chain starts before the 64-row block copy.
                    avs = xpool.tile([DH + 1, N], BF16, name=f"avs{h}", tag="avs")
                    nc.vector.tensor_copy(
                        avs[DH : DH + 1, :], avs_t[hh][DH : DH + 1, :]
                    )
                    # Head hh's 1024 sums land as 4 chunks of 256 on
                    # partitions 4*hh .. 4*hh+3 (chunk-major fill).
                    nc.sync.dma_start(
                        out=s2s[p][4 * hh : 4 * hh + 4, :],
                        in_=avs[DH : DH + 1, :],
                    )
                    nc.vector.tensor_copy(avs[0:DH, :], avs_t[hh][0:DH, :])
                    nc.sync.dma_start(out=outTs[p][hs, :], in_=avs[0:DH, :])

                for hh in range(2):
                    avs_t[hh] = pspool.tile(
                        [DH + 1, N], F32, name=f"av{2*p+hh}", tag="ps"
                    )
                    # head 0 drives pair p+1's q projection, head 1 its k
                    # projection: matmuls slotted into each jt step.
                    proj = (
                        proj_steps(p + 1, hh, pspool, "ps")
                        if p + 1 < PAIRS
                        else iter(())
                    )

                    def proj_step(k=3):
                        for _ in range(k):
                            next(proj, None)

                    emit_sim(hh, 0)
                    proj_step()
                    for jt in range(1, NT):
                        emit_sim(hh, jt)
                        emit_av(hh, jt - 1)
                        proj_step()
                        if hh == 1 and jt == 3 and p > 0:
                            emit_norm(p - 1)
                    emit_av(hh, NT - 1)
                    proj_step(1)  # run the projection eviction
                    emit_evict(hh)

            # ---- Phase F: y = outT^T @ w_out ------------------------------
            # Three psy groups accumulate kt=0..6 (21 matmuls, ~4.5us) while
            # pair 7's reciprocal chain completes, then its norm runs, then
            # the held kt=7 contributions — the PE never idles >3.4us, so no
            # HAM re-throttle into the tail. (Only 3 groups: the norm's rs
            # tile needs the 4th PSUM slot.)
            psys = {}
            ysts = {}

            def psy_mm(it, hf, kt):
                sl = slice(hf * 512, hf * 512 + 512)
                nc.tensor.matmul(
                    psys[(it, hf)],
                    lhsT=outTs[kt][:, it * P : (it + 1) * P],
                    rhs=wo_t[:, kt, sl],
                    start=(kt == 0),
                    stop=(kt == KT - 1),
                )

            prelude = [(0, 0), (0, 1), (1, 0)]
            for it in range(2):
                ysts[it] = xpool.tile([P, DIM], F32, name=f"y{it}", tag="x")
            for it, hf in prelude:
                psys[(it, hf)] = pspool.tile(
                    [P, 512], F32, name=f"psy{it}_{hf}", tag="ps"
                )
                for kt in range(KT - 1):
                    psy_mm(it, hf, kt)
            emit_norm(PAIRS - 1)
            for it, hf in prelude:
                psy_mm(it, hf, KT - 1)
                nc.vector.tensor_copy(
                    ysts[it][:, hf * 512 : hf * 512 + 512], psys[(it, hf)]
                )
            for it in range(NT):
                if it >= 2:
                    ysts[it] = xpool.tile([P, DIM], F32, name=f"y{it}", tag="x")
                for hf in range(2):
                    if (it, hf) in psys:
                        continue
                    sl = slice(hf * 512, hf * 512 + 512)
                    psys[(it, hf)] = pspool.tile(
                        [P, 512], F32, name=f"psy{it}_{hf}", tag="ps"
                    )
                    for kt in range(KT):
                        psy_mm(it, hf, kt)
                    nc.vector.tensor_copy(ysts[it][:, sl], psys[(it, hf)])
                nc.sync.dma_start(out=y_d[it * P : (it + 1) * P, :], in_=ysts[it])

    nc.compile()
    _BUILD_CACHE[key] = nc
    return nc


def _host_prep(ln_gamma, ln_beta, w_qkv, w_out, attn_bias):
    """Re-layout weights/bias for the device kernel (pure host-side reshapes)."""
    w_qkv = np.asarray(w_qkv, np.float32)
    w_out = np.asarray(w_out, np.float32)
    attn_bias = np.asarray(attn_bias, np.float32)

    wq_r = w_qkv[:, :INNER].reshape(KT, P, PAIRS, P).transpose(2, 1, 0, 3)
    wk_r = w_qkv[:, INNER : 2 * INNER].reshape(KT, P, PAIRS, P).transpose(2, 1, 0, 3)
    wqk = np.ascontiguousarray(
        np.concatenate([wq_r, wk_r], axis=3).astype(ml_dtypes.bfloat16)
    )
    wv = np.ascontiguousarray(
        w_qkv[:, 2 * INNER :].reshape(KT, P, DIM).transpose(1, 0, 2)
        .astype(ml_dtypes.bfloat16)
    )
    wo = np.ascontiguousarray(
        w_out.reshape(KT, P, DIM).transpose(1, 0, 2).astype(ml_dtypes.bfloat16)
    )
    # exp(bias), transposed per head to [j, i]: the kernel multiplies it into
    # exp(sim) on the vector engine (exp(a+b) = exp(a)*exp(b)), keeping the
    # tensor engine free of bias-add matmuls.
    biasT = np.ascontiguousarray(
        np.exp(attn_bias[0].astype(np.float64)).astype(np.float32)
        .transpose(0, 2, 1)
        .reshape(HEADS, NT, P, N)
        .astype(ml_dtypes.bfloat16)
    )
    sel8 = np.zeros((8, 4, P), dtype=ml_dtypes.bfloat16)
    for c in range(4):
        sel8[c, c, 0:DH] = 1.0        # head 0 of the pair -> outT rows 0:64
        sel8[4 + c, c, DH:P] = 1.0    # head 1 of the pair -> outT rows 64:128
    in_map = {"wqk": wqk, "wv": wv, "wo": wo, "biasT": biasT, "sel8": sel8}

    gamma = np.asarray(ln_gamma, np.float32)
    beta = np.asarray(ln_beta, np.float32)
    apply_gamma = not np.all(gamma == 1.0)
    apply_beta = bool(np.any(beta != 0.0))
    if apply_gamma:
        in_map["gamma"] = gamma
    if apply_beta:
        in_map["beta"] = beta
    return in_map, apply_gamma, apply_beta


def kernel(x, ln_gamma, ln_beta, w_qkv, w_out, attn_bias):
    x = np.asarray(x, np.float32)
    in_map, apply_gamma, apply_beta = _host_prep(
        ln_gamma, ln_beta, w_qkv, w_out, attn_bias
    )
    nc = _build(apply_gamma, apply_beta)
    in_maps = [dict(in_map, x=np.ascontiguousarray(x[b])) for b in range(B)]
    res = run_bass_kernel_spmd(
        nc,
        in_maps,
        list(range(B)),
        trace=bool(int(os.environ.get("BA_TRACE", "0"))),
        tmpdir=os.environ.get("BA_TRACE_DIR") or None,
    )
    out = np.stack([res.results[i]["y"] for i in range(B)], axis=0)
    if bool(int(os.environ.get("BA_TRACE", "0"))):
        kernel.last_exec_time_ns = res.exec_time_ns
        kernel.last_mean_exec_time_ns = res.mean_exec_time_ns
    return out
